# revision 46
# baseline (speedup 1.0000x reference)
"""Trainium2 Bass kernel for nn_Bert_44452911514066 (DeBERTa-style disentangled
attention BERT layer), data-parallel over batch across 8 NeuronCores.

kernel(**inputs) takes the FULL inputs (as produced by reference.setup_inputs)
and returns the FULL [S, B, H] output.

Key ideas:
  - batch-DP: 2 batches per core, weights/tables replicated.
  - the relative-position gather is Toeplitz: per (b,h), bucket values are
    expanded into "diagonal space" by matmuls (rhs = per-head M matrices built
    from a one-hot bucket expansion); the diagonal shear is applied by DMAs
    whose access pattern steps (partition+1, elem-1) over the window tiles.
  - scores are assembled transposed [k, q] in PSUM: CC matmul + identity
    matmuls of the sheared cq/ck tiles. The cq tiles (natural [q, k]) are
    sheared AND transposed in a single xbar-transpose DMA (stride 656 rows).
    No PE transposes in the attention loop -> the PE HAM stays at K=8/8.
  - softmax without max-subtraction: exp(s - 12) on ScalarE; masking and the
    denominator are folded into an augmented/masked V matrix. PV runs with V
    stationary into one [65, 512] PSUM bank; the ctx^T result is transposed
    back by xbar DMAs and divided on DVE.
  - LN1/LN2 transposes are xbar DMAs per 128-token tile (no DRAM roundtrip).
  - fp16 matmul inputs everywhere (full PE rate), fp32 accumulation.
"""
import sys
sys.path.insert(0, "/opt/trn_rl_repo")
import math
import functools
import contextlib
import numpy as np

import concourse.bass as bass
import concourse.tile as tile
from concourse import mybir
from concourse.masks import make_identity

H, NH, HD, S, B = 768, 12, 64, 512, 16
NCORES = 8
BL = B // NCORES          # batches per core
T = BL * S                # tokens per core
SCALE = 1.0 / math.sqrt(3 * HD)
EPS = 1e-7
NB = 63                   # relative buckets
WIN = 657                 # window elems per row; stride WIN-1=656 is 32B-aligned
OFF0 = 128                # shear: window col j = OFF0 + k - q
EW = 1032                 # padded E-table width (zero col + 1024 data + pad)
CSHIFT = 12.0             # exp shift
F16 = mybir.dt.float16
F32 = mybir.dt.float32
AF = mybir.ActivationFunctionType
OP = mybir.AluOpType

# ---------------------------------------------------------------------------
# walrus workaround: this container's walrus accepts at most ONE sync wait per
# instruction; split extra waits onto single-wait NoOps.
# ---------------------------------------------------------------------------
from concourse.vector_clock import ScopedClock

_orig_add_instruction = tile.TileContext._add_instruction


def _patched_add_instruction(self, inst):
    si = inst.sync_info
    if si is not None and si.on_wait is not None and len(si.on_wait) > 1:
        waits = list(si.on_wait)
        for i, w in enumerate(waits[:-1]):
            nop = mybir.InstNoOp(name=f"{inst.name}-wsplit{i}", ins=[], outs=[])
            nop.engine = inst.engine
            nop.sync_info = mybir.SyncInfo(on_wait=[w], on_update=[])
            _orig_add_instruction(self, nop)
        inst.sync_info = mybir.SyncInfo(
            on_wait=[waits[-1]], on_update=list(si.on_update or []))
    _orig_add_instruction(self, inst)


def _patched_drain_and_barrier(self, tick_clock, wait_clock):
    nc = self.nc
    probe = nc.sync.nop(nofuse=True)
    wait_clock.add_sem_waits(probe.ins, ScopedClock({None: tick_clock.global_clock}))
    si = probe.ins.sync_info
    waits = list(si.on_wait) if si is not None and si.on_wait else []
    if len(waits) > 1:
        probe.ins.sync_info = mybir.SyncInfo(on_wait=waits[:1], on_update=[])
        for w in waits[1:]:
            n2 = nc.sync.nop(nofuse=True)
            n2.ins.sync_info = mybir.SyncInfo(on_wait=[w], on_update=[])
    nc.sync.drain()
    nc.all_engine_barrier()
    assert self.sems is not None
    popped = nc._tile_sem_poison_stack.pop()
    assert popped is self._sem_poison
    nc.clear_and_free_semaphores(list(self.sems.allocated().values()))
    nc.all_engine_barrier()


tile.TileContext._add_instruction = _patched_add_instruction
tile.TileContext._drain_and_barrier = _patched_drain_and_barrier


def _shear_ap(t_ap, ncols, pitch=WIN):
    """out[p, j] = flat[p*(pitch-1) + base + OFF0 + j]: per-partition start
    shifts back one element per row, staying inside each row's window."""
    return bass.AP(tensor=t_ap.tensor, offset=t_ap.offset + OFF0,
                   ap=[[pitch - 1, 128], [1, ncols]])


_FULL_TRANSPOSE = True
_PAIR_INTERLEAVE = False
MMLOG = {}  # mybir instruction name -> human label (for trace analysis)


def _mm(nc, label, *args, **kwargs):
    r = nc.tensor.matmul(*args, **kwargs)
    MMLOG[r.ins.name] = label
    return r


# ---------------------------------------------------------------------------
# device kernel build
# ---------------------------------------------------------------------------
@functools.lru_cache(maxsize=2)
def build_module(with_bias: bool):
    nc = bass.Bass()

    hid_d = nc.dram_tensor("hid", [T, H], F32, kind="ExternalInput")
    wqkT_d = nc.dram_tensor("wqkT", [H, 2 * H], F16, kind="ExternalInput")
    wvgT_d = nc.dram_tensor("wvgT", [H, 2 * H], F16, kind="ExternalInput")
    woutT_d = nc.dram_tensor("woutT", [H, H], F16, kind="ExternalInput")
    relT_d = nc.dram_tensor("relT", [H, NB], F16, kind="ExternalInput")
    Ecq_d = nc.dram_tensor("Ecq", [NB, EW], F16, kind="ExternalInput")
    Eck_d = nc.dram_tensor("Eck", [NB, EW], F16, kind="ExternalInput")
    vmask_d = nc.dram_tensor("vmask", [T, 1], F32, kind="ExternalInput")
    if with_bias:
        # host-prepared: bqkc[p, f] = b_qk[128f+p] * (SCALE if f<6 else 1)
        bqkc_d = nc.dram_tensor("bqkc", [128, 12], F32, kind="ExternalInput")
        # rows replicated for free-dim adds
        bqkr_d = nc.dram_tensor("bqkr", [1, 2 * H], F32, kind="ExternalInput")
        bvgr_d = nc.dram_tensor("bvgr", [1, 2 * H], F32, kind="ExternalInput")
        boutr_d = nc.dram_tensor("boutr", [1, H], F32, kind="ExternalInput")
    out_d = nc.dram_tensor("out", [T, H], F32, kind="ExternalOutput")

    with tile.TileContext(nc) as tc, contextlib.ExitStack() as ctx:
        persist = ctx.enter_context(tc.tile_pool(name="persist", bufs=1))
        stats = ctx.enter_context(tc.tile_pool(name="stats", bufs=4))

        # --- constants ---
        ident16 = persist.tile([128, 128], F16, tag="id16")
        make_identity(nc, ident16)
        eps_t = persist.tile([128, 1], F32, tag="eps")
        nc.vector.memset(eps_t, EPS)
        negc_t = persist.tile([128, 1], F32, tag="negc")
        nc.vector.memset(negc_t, -CSHIFT)

        # --- load weights / tables ---
        # trans pool holds tables only needed through phase 2; closed before
        # the attention pools open so its SBUF is reclaimed.
        trans_stack = contextlib.ExitStack()
        trans = trans_stack.enter_context(tc.tile_pool(name="trans", bufs=1))
        wqkT = persist.tile([128, 6, 2 * H], F16, tag="wqkT")
        wvgT = persist.tile([128, 6, 2 * H], F16, tag="wvgT")
        woutT = persist.tile([128, 6, H], F16, tag="woutT")
        relT = trans.tile([128, 6, NB], F16, tag="relT")
        for c in range(6):
            nc.sync.dma_start(out=wqkT[:, c, :], in_=wqkT_d[128 * c:128 * c + 128, :])
            nc.sync.dma_start(out=relT[:, c, :], in_=relT_d[128 * c:128 * c + 128, :])
        Ecq = trans.tile([NB, EW], F16, tag="Ecq")
        Eck = trans.tile([NB, EW], F16, tag="Eck")
        nc.sync.dma_start(out=Ecq[:], in_=Ecq_d[:])
        nc.sync.dma_start(out=Eck[:], in_=Eck_d[:])
        for c in range(6):
            nc.sync.dma_start(out=wvgT[:, c, :], in_=wvgT_d[128 * c:128 * c + 128, :])
        vmask16 = trans.tile([128, 8], F32, tag="vm")
        nc.sync.dma_start(
            out=vmask16[:],
            in_=vmask_d[:].rearrange("(t p) one -> p (t one)", p=128))
        if with_bias:
            bqkc = persist.tile([128, 12], F32, tag="bqkc")
            nc.sync.dma_start(out=bqkc[:], in_=bqkc_d[:])
            bqkr = persist.tile([64, 2 * H], F32, tag="bqkr")
            nc.sync.dma_start(
                out=bqkr[:],
                in_=bass.AP(tensor=bqkr_d, offset=0, ap=[[0, 64], [1, 2 * H]]))
            bvgr = persist.tile([128, 2 * H], F32, tag="bvgr")
            nc.sync.dma_start(
                out=bvgr[:],
                in_=bass.AP(tensor=bvgr_d, offset=0, ap=[[0, 128], [1, 2 * H]]))
            boutr = persist.tile([128, H], F32, tag="boutr")
            nc.sync.dma_start(
                out=boutr[:],
                in_=bass.AP(tensor=boutr_d, offset=0, ap=[[0, 128], [1, H]]))
        for c in range(6):
            nc.sync.dma_start(out=woutT[:, c, :], in_=woutT_d[128 * c:128 * c + 128, :])

        def layernorm_to(out16, xin, tag):
            st = stats.tile([128, 3, 6], F32, tag="bnst")
            for sg in range(3):
                nc.vector.bn_stats(out=st[:, sg, :], in_=xin[:, 256 * sg:256 * sg + 256])
            mv = stats.tile([128, 2], F32, tag="bnmv")
            nc.vector.bn_aggr(out=mv[:], in_=st[:])
            rstd = stats.tile([128, 1], F32, tag="rstd")
            nc.scalar.activation(out=rstd[:], in_=mv[:, 1:2], func=AF.Sqrt,
                                 bias=eps_t[:], scale=1.0)
            nc.vector.reciprocal(out=rstd[:], in_=rstd[:])
            nc.vector.scalar_tensor_tensor(
                out=out16, in0=xin, scalar=mv[:, 0:1],
                in1=rstd[:].to_broadcast((128, H)),
                op0=OP.subtract, op1=OP.mult)

        # --- pos projection + M matrices (PE warms up on these) ---
        posp = trans.tile([64, 2 * H], F16, tag="posp")
        Mh = persist.tile([128, 6, EW], F16, tag="Mh")
        Mq = persist.tile([128, 6, EW], F16, tag="Mq")
        qk16 = persist.tile([128, 12, T], F16, tag="qk16")
        g16 = persist.tile([128, 8, H], F16, tag="g16")
        va16 = persist.tile([128, 8, NH * 65], F16, tag="va16")
        hT = persist.tile([128, 6, T], F16, tag="hT")
        ln2T = hT  # reused after QK/VG consume hT
        ctx16 = persist.tile([128, 8, H], F16, tag="ctx16")

        with tc.tile_pool(name="ph2ps", bufs=4, space="PSUM") as ph2ps, \
             tc.tile_pool(name="ph12", bufs=2) as ph12:
            # pos projection (only needs relT + wqkT)
            for fc in range(3):
                ps = ph2ps.tile([128, 512], F32, tag="ps2")
                for c in range(6):
                    nc.tensor.matmul(
                        ps[:NB, :], relT[:, c, :], wqkT[:, c, 512 * fc:512 * fc + 512],
                        start=(c == 0), stop=(c == 5))
                if fc == 0:
                    segs = [(0, 512, SCALE)]
                elif fc == 1:
                    segs = [(0, 256, SCALE), (256, 512, 1.0)]
                else:
                    segs = [(0, 512, 1.0)]
                for (a, b_, sc) in segs:
                    if with_bias:
                        nc.vector.scalar_tensor_tensor(
                            out=posp[:NB, 512 * fc + a:512 * fc + b_],
                            in0=ps[:NB, a:b_], scalar=float(sc),
                            in1=bqkr[:NB, 512 * fc + a:512 * fc + b_],
                            op0=OP.mult, op1=OP.add)
                    else:
                        nc.vector.tensor_scalar_mul(
                            out=posp[:NB, 512 * fc + a:512 * fc + b_],
                            in0=ps[:NB, a:b_], scalar1=float(sc))
            # M matrices (per head pair; odd head in partitions 64-127).
            # Only the first 1024 columns of the EW-wide tables are ever read.
            for p in range(6):
                for half in range(2):
                    hh = 2 * p + half
                    r0 = 64 * half
                    for ec in range(2):
                        ps = ph2ps.tile([128, 512], F32, tag="ps2")
                        nc.tensor.matmul(
                            ps[r0:r0 + 64, :],
                            posp[:NB, H + 64 * hh:H + 64 * hh + 64],
                            Ecq[:, 512 * ec:512 * ec + 512],
                            start=True, stop=True, tile_position=(0, r0))
                        nc.scalar.activation(
                            out=Mh[r0:r0 + 64, p, 512 * ec:512 * ec + 512],
                            in_=ps[r0:r0 + 64, :], func=AF.Copy)
                        ps2 = ph2ps.tile([128, 512], F32, tag="ps2")
                        nc.tensor.matmul(
                            ps2[r0:r0 + 64, :],
                            posp[:NB, 64 * hh:64 * hh + 64],
                            Eck[:, 512 * ec:512 * ec + 512],
                            start=True, stop=True, tile_position=(0, r0))
                        nc.vector.tensor_copy(
                            out=Mq[r0:r0 + 64, p, 512 * ec:512 * ec + 512],
                            in_=ps2[r0:r0 + 64, :])

            # --- phase 1: LN1 per tile -> h16 -> xbar-transpose into hT ---
            for t in range(8):
                xt = ph12.tile([128, H], F32, tag="x")
                nc.sync.dma_start(out=xt[:], in_=hid_d[128 * t:128 * t + 128, :])
                h16 = ph12.tile([128, H], F16, tag="h16")
                layernorm_to(h16[:], xt[:], f"ln1_{t}")
                nc.sync.dma_start(out=hT[:, :, 128 * t:128 * t + 128],
                                  in_=h16[:], transpose=True)

            # --- phase 2: projections ---
            def vg_tile(t):
                vg_t = ph12.tile([128, 2 * H], F16, tag="vg")
                for fc in range(3):
                    ps = ph2ps.tile([128, 512], F32, tag="ps2")
                    for c in range(6):
                        nc.tensor.matmul(
                            ps[:], hT[:, c, 128 * t:128 * t + 128],
                            wvgT[:, c, 512 * fc:512 * fc + 512],
                            start=(c == 0), stop=(c == 5))
                    if with_bias:
                        nc.vector.scalar_tensor_tensor(
                            out=vg_t[:, 512 * fc:512 * fc + 512], in0=ps[:], scalar=1.0,
                            in1=bvgr[:, 512 * fc:512 * fc + 512],
                            op0=OP.mult, op1=OP.add)
                    else:
                        nc.vector.tensor_copy(
                            out=vg_t[:, 512 * fc:512 * fc + 512], in_=ps[:])
                nc.scalar.activation(out=g16[:, t, :], in_=vg_t[:, H:2 * H], func=AF.Gelu)
                for hh in range(NH):
                    nc.vector.tensor_scalar_mul(
                        out=va16[:, t, 65 * hh:65 * hh + 64],
                        in0=vg_t[:, 64 * hh:64 * hh + 64],
                        scalar1=vmask16[:, t:t + 1])
                vav = va16[:, t, :].rearrange("p (h c) -> p h c", h=NH)
                nc.vector.tensor_copy(
                    out=vav[:, :, 64],
                    in_=vmask16[:, t:t + 1].to_broadcast((128, NH)))

            def qk_half(nh):
                for f in range(12):
                    ps = ph2ps.tile([128, 512], F32, tag="ps2")
                    for c in range(6):
                        nc.tensor.matmul(
                            ps[:], wqkT[:, c, 128 * f:128 * f + 128],
                            hT[:, c, 512 * nh:512 * nh + 512],
                            start=(c == 0), stop=(c == 5))
                    if with_bias:
                        nc.scalar.activation(
                            out=qk16[:, f, 512 * nh:512 * nh + 512], in_=ps[:],
                            func=AF.Identity, bias=bqkc[:, f:f + 1],
                            scale=SCALE if f < 6 else 1.0)
                    else:
                        nc.scalar.activation(
                            out=qk16[:, f, 512 * nh:512 * nh + 512], in_=ps[:],
                            func=AF.Copy, bias=0.0,
                            scale=SCALE if f < 6 else 1.0)

            for t in range(4):
                vg_tile(t)
            qk_half(0)
            for t in range(4, 8):
                vg_tile(t)
            qk_half(1)
        trans_stack.close()

        # --- phase 3 attention + phase 4 epilogue, per batch ---
        with tc.tile_pool(name="wps", bufs=1, space="PSUM") as wps, \
             tc.tile_pool(name="scps", bufs=2, space="PSUM") as scps, \
             tc.tile_pool(name="pvps", bufs=2, space="PSUM") as pvps, \
             tc.tile_pool(name="shear", bufs=2) as shp, \
             tc.tile_pool(name="etp", bufs=4) as etp, \
             tc.tile_pool(name="ph4", bufs=2) as ph4:
            def emit_expansions(b, hh):
                tok0 = 512 * b
                p, half = hh // 2, hh % 2
                r0 = 64 * half
                cqT = shp.tile([128, 4, 4, 128], F16, tag="cqT", bufs=3)   # [kl, t, u, q]
                cqsh = shp.tile([128, 4, 512], F16, tag="cqsh")    # [q, qt, k]
                cksh = shp.tile([128, 4, 512], F16, tag="cksh", bufs=3)    # [kl, kt, q]
                for t in range(4):
                    ws = 384 - 128 * t
                    lq = qk16[r0:r0 + 64, p, tok0 + 128 * t:tok0 + 128 * t + 128]
                    lk = qk16[r0:r0 + 64, 6 + p, tok0 + 128 * t:tok0 + 128 * t + 128]
                    wq = shp.tile([128, WIN], F16, tag="wcq")
                    wk = shp.tile([128, WIN], F16, tag="wck")
                    pa = wps.tile([128, 512], F32, tag="wpsa", bufs=2)
                    pbp = wps.tile([128, 256], F32, tag="wpsb", bufs=1)
                    _mm(nc, f'exp-qa-{t}', pa[:], lq, Mh[r0:r0 + 64, p, ws:ws + 512],
                        start=True, stop=True)
                    _mm(nc, f'exp-qb-{t}', pbp[:, 0:128], lq,
                        Mh[r0:r0 + 64, p, ws + 512:ws + 640], start=True, stop=True)
                    nc.scalar.activation(out=wq[:, :256], in_=pa[:, :256],
                                         func=AF.Copy)
                    nc.vector.tensor_copy(out=wq[:, 256:512], in_=pa[:, 256:512])
                    nc.scalar.activation(out=wq[:, 512:640], in_=pbp[:, 0:128], func=AF.Copy)
                    pa2 = wps.tile([128, 512], F32, tag="wpsa2", bufs=2)
                    _mm(nc, f'exp-ka-{t}', pa2[:], lk, Mq[r0:r0 + 64, p, ws:ws + 512],
                        start=True, stop=True)
                    _mm(nc, f'exp-kb-{t}', pbp[:, 128:256], lk,
                        Mq[r0:r0 + 64, p, ws + 512:ws + 640], start=True, stop=True)
                    nc.vector.tensor_copy(out=wk[:, :256], in_=pa2[:, :256])
                    nc.scalar.activation(out=wk[:, 256:512], in_=pa2[:, 256:512],
                                         func=AF.Copy)
                    nc.scalar.activation(out=wk[:, 512:640], in_=pbp[:, 128:256],
                                         func=AF.Copy)
                    # plain shears on the gpsimd swdge queue
                    nc.gpsimd.dma_start(out=cqsh[:, t, :], in_=_shear_ap(wq[:], 512))
                    nc.gpsimd.dma_start(out=cksh[:, t, :], in_=_shear_ap(wk[:], 512))
                # one xbar-transpose for the whole head: [q,(t,k)] -> [kl,(t,u),q]
                nc.sync.dma_start(out=cqT[:], in_=cqsh[:], transpose=True)
                return cqT, cksh

            def emit_scores_pv(b, hh, cqT, cksh):
                tok0 = 512 * b
                p, half = hh // 2, hh % 2
                r0 = 64 * half
                cpsT = pvps.tile([80, 512], F32, tag="cpsT", bufs=1)
                va_h = va16[:, :, 65 * hh:65 * hh + 65]
                ets = []

                def pv_mms(u):
                    for t in range(4):
                        _mm(nc, f'pv-{u}-{t}',
                            cpsT[:65, 128 * t:128 * t + 128],
                            va_h[:, 4 * b + u, :],
                            ets[u][:, 128 * t:128 * t + 128],
                            start=(u == 0 and t == 0), stop=(u == 3 and t == 3))

                for u in range(4):
                    sc = scps.tile([128, 512], F32, tag="sc")
                    _mm(nc, f'sc-cc-{u}',
                        sc[:],
                        qk16[r0:r0 + 64, 6 + p, tok0 + 128 * u:tok0 + 128 * u + 128],
                        qk16[r0:r0 + 64, p, tok0:tok0 + 512],
                        start=True, stop=False)
                    _mm(nc, f'sc-ck-{u}', sc[:], ident16[:], cksh[:, u, :],
                                     start=False, stop=False)
                    _mm(nc, f'sc-cq-{u}', sc[:], ident16[:], cqT[:, :, u, :],
                                     start=False, stop=True)
                    e_u = etp.tile([128, 512], F16, tag="et")
                    nc.scalar.activation(out=e_u[:], in_=sc[:], func=AF.Exp,
                                         bias=negc_t[:], scale=1.0)
                    ets.append(e_u)
                    pv_mms(u)
                # -- evict ctx^T; transpose + divide deferred one cycle --
                cps16 = shp.tile([80, 512], F16, tag="cps16", bufs=3)
                if emit_scores_pv.n < 3:
                    nc.vector.memset(cps16[64:80, :], 0.0)
                emit_scores_pv.n += 1
                nc.vector.tensor_copy(out=cps16[:65, :], in_=cpsT[:65, :])
                return cps16

            def emit_ctx_finish(b, hh, cps16):
                ctxq = shp.tile([128, 4, 80], F16, tag="ctxq")
                nc.sync.dma_start(out=ctxq[:], in_=cps16[:80, :], transpose=True)
                rec4 = stats.tile([128, 4], F32, tag="rec")
                nc.vector.reciprocal(out=rec4[:], in_=ctxq[:, :, 64])
                for t in range(4):
                    nc.vector.tensor_scalar_mul(
                        out=ctx16[:, 4 * b + t, 64 * hh:64 * hh + 64],
                        in0=ctxq[:, t, 0:64], scalar1=rec4[:, t:t + 1])

            emit_scores_pv.n = 0

            def emit_p4(b):
                # --- phase 4 for this batch: gate, LN2, transpose, out proj ---
                for g in range(4 * b, 4 * b + 4):
                    cg = ph4.tile([128, H], F16, tag="cg")
                    nc.vector.tensor_mul(cg[:], ctx16[:, g, :], g16[:, g, :])
                    ln2 = ph4.tile([128, H], F16, tag="ln2")
                    layernorm_to(ln2[:], cg[:], f"ln2_{g}")
                    nc.sync.dma_start(out=ln2T[:, :, 128 * g:128 * g + 128],
                                      in_=ln2[:], transpose=True)
                for g in range(4 * b, 4 * b + 4):
                    ot = ph4.tile([128, H], F32, tag="ot")  # noqa: indent kept
                    for fc, (f0, fw) in enumerate([(0, 512), (512, 256)]):
                        ps = scps.tile([128, 512], F32, tag="sc")
                        for c in range(6):
                            nc.tensor.matmul(
                                ps[:, :fw], ln2T[:, c, 128 * g:128 * g + 128],
                                woutT[:, c, f0:f0 + fw],
                                start=(c == 0), stop=(c == 5))
                        if with_bias:
                            nc.vector.scalar_tensor_tensor(
                                out=ot[:, f0:f0 + fw], in0=ps[:, :fw], scalar=1.0,
                                in1=boutr[:, f0:f0 + fw], op0=OP.mult, op1=OP.add)
                        else:
                            nc.vector.tensor_copy(out=ot[:, f0:f0 + fw], in_=ps[:, :fw])
                    nc.sync.dma_start(out=out_d[128 * g:128 * g + 128, :], in_=ot[:])

            # software-pipelined driver: head i+1's expansions are emitted
            # before head i's score/PV matmuls so the shear+transpose DMA
            # latency hides behind real PE work.
            from collections import deque
            work = [(b, hh) for b in range(BL) for hh in range(NH)]
            pend = deque()
            fin = deque()
            def drain_fin():
                (fb_, fh_), fcps = fin.popleft()
                emit_ctx_finish(fb_, fh_, fcps)
                if fh_ == NH - 1:
                    emit_p4(fb_)
            def drain_one():
                (pb_, ph_), (pcqT, pcksh) = pend.popleft()
                cps = emit_scores_pv(pb_, ph_, pcqT, pcksh)
                fin.append(((pb_, ph_), cps))
                if len(fin) > 1:
                    drain_fin()
            for w in work:
                tiles = emit_expansions(*w)
                pend.append((w, tiles))
                if len(pend) > 2:
                    drain_one()
            while pend:
                drain_one()
            while fin:
                drain_fin()

    return nc


# ---------------------------------------------------------------------------
# host side
# ---------------------------------------------------------------------------
def _host_prep(position_indices, attention_mask):
    pi = np.asarray(position_indices)
    gvec = np.empty(1023, np.int64)
    gvec[511:] = pi[:, 0]
    gvec[:512] = pi[0, ::-1]
    d = np.arange(S)[:, None] - np.arange(S)[None, :]
    assert np.array_equal(gvec[d + 511], pi), "position_indices not Toeplitz"
    e = np.arange(1023)
    # table col c maps to window col j = c - ws; one zero col prepended so the
    # t=3 window slice starts at col 0 (cq: c = 512 - delta, ck: c = 512 + delta)
    E_cq = np.zeros((NB, EW), np.float16)
    E_ck = np.zeros((NB, EW), np.float16)
    E_cq[:, 1:1024] = (np.arange(NB)[:, None] == gvec[1022 - e][None, :])
    E_ck[:, 1:1024] = (np.arange(NB)[:, None] == gvec[e][None, :])
    am = np.asarray(attention_mask).reshape(B, S)
    vmask = (~am).astype(np.float32)
    return E_cq, E_ck, vmask


def kernel(hidden_states, relative_embedding, w_qk, b_qk, w_vg, b_vg,
           w_out, b_out, attention_mask, position_indices):
    from concourse.bass_utils import run_bass_kernel_spmd

    hidden_states = np.asarray(hidden_states, dtype=np.float32)
    relative_embedding = np.asarray(relative_embedding, dtype=np.float32)
    w_qk = np.asarray(w_qk, dtype=np.float32)
    w_vg = np.asarray(w_vg, dtype=np.float32)
    w_out = np.asarray(w_out, dtype=np.float32)
    b_qk = np.asarray(b_qk, dtype=np.float32)
    b_vg = np.asarray(b_vg, dtype=np.float32)
    b_out = np.asarray(b_out, dtype=np.float32)

    with_bias = bool(np.any(b_qk) or np.any(b_vg) or np.any(b_out))
    E_cq, E_ck, vmask = _host_prep(position_indices, attention_mask)

    nc = build_module(with_bias)
    common = dict(
        wqkT=np.ascontiguousarray(w_qk.T).astype(np.float16),
        wvgT=np.ascontiguousarray(w_vg.T).astype(np.float16),
        woutT=np.ascontiguousarray(w_out.T).astype(np.float16),
        relT=np.ascontiguousarray(relative_embedding.T).astype(np.float16),
        Ecq=E_cq, Eck=E_ck)
    if with_bias:
        sc_col = np.where(np.arange(12) < 6, SCALE, 1.0).astype(np.float32)
        common["bqkc"] = np.ascontiguousarray(
            b_qk.reshape(12, 128).T * sc_col[None, :])
        sc_row = np.concatenate([np.full(H, SCALE), np.ones(H)]).astype(np.float32)
        common["bqkr"] = (b_qk * sc_row)[None, :].astype(np.float32)
        common["bvgr"] = b_vg[None, :].astype(np.float32)
        common["boutr"] = b_out[None, :].astype(np.float32)

    in_maps = []
    for core in range(NCORES):
        bsel = [BL * core + i for i in range(BL)]
        hid = np.ascontiguousarray(
            hidden_states[:, bsel, :].transpose(1, 0, 2).reshape(T, H))
        vm = np.ascontiguousarray(vmask[bsel].reshape(T, 1))
        in_maps.append(dict(common, hid=hid, vmask=vm))

    res = run_bass_kernel_spmd(nc, in_maps, list(range(NCORES)))
    out = np.empty((S, B, H), np.float32)
    for core in range(NCORES):
        o = res.results[core]["out"].reshape(BL, S, H)
        for i in range(BL):
            out[:, BL * core + i, :] = o[i]
    return out


# revision 47
# speedup vs baseline: 1.1349x; 1.1349x over previous
"""Trainium2 Bass kernel for nn_Bert_44452911514066 (DeBERTa-style disentangled
attention BERT layer), data-parallel over batch across 8 NeuronCores.

kernel(**inputs) takes the FULL inputs (as produced by reference.setup_inputs)
and returns the FULL [S, B, H] output.

Key ideas:
  - batch-DP: 2 batches per core, weights/tables replicated.
  - the relative-position gather is Toeplitz: per (b,h), bucket values are
    expanded into "diagonal space" by matmuls (rhs = per-head M matrices built
    from a one-hot bucket expansion); the diagonal shear is applied by DMAs
    whose access pattern steps (partition+1, elem-1) over the window tiles.
  - scores are assembled transposed [k, q] in PSUM: CC matmul + identity
    matmuls of the sheared cq/ck tiles. The cq tiles (natural [q, k]) are
    sheared AND transposed in a single xbar-transpose DMA (stride 656 rows).
    No PE transposes in the attention loop -> the PE HAM stays at K=8/8.
  - softmax without max-subtraction: exp(s - 12) on ScalarE; masking and the
    denominator are folded into an augmented/masked V matrix. PV runs with V
    stationary into one [65, 512] PSUM bank; the ctx^T result is transposed
    back by xbar DMAs and divided on DVE.
  - LN1/LN2 transposes are xbar DMAs per 128-token tile (no DRAM roundtrip).
  - fp16 matmul inputs everywhere (full PE rate), fp32 accumulation.
"""
import sys
sys.path.insert(0, "/opt/trn_rl_repo")
import math
import functools
import contextlib
import numpy as np

import concourse.bass as bass
import concourse.tile as tile
from concourse import mybir
from concourse.masks import make_identity

H, NH, HD, S, B = 768, 12, 64, 512, 16
NCORES = 8
BL = B // NCORES          # batches per core
T = BL * S                # tokens per core
SCALE = 1.0 / math.sqrt(3 * HD)
EPS = 1e-7
NB = 63                   # relative buckets
WIN = 657                 # window elems per row; stride WIN-1=656 is 32B-aligned
OFF0 = 128                # shear: window col j = OFF0 + k - q
EW = 1032                 # padded E-table width (zero col + 1024 data + pad)
CSHIFT = 12.0             # exp shift
F16 = mybir.dt.float16
F32 = mybir.dt.float32
AF = mybir.ActivationFunctionType
OP = mybir.AluOpType

# ---------------------------------------------------------------------------
# walrus workaround: this container's walrus accepts at most ONE sync wait per
# instruction; split extra waits onto single-wait NoOps.
# ---------------------------------------------------------------------------
from concourse.vector_clock import ScopedClock

_orig_add_instruction = tile.TileContext._add_instruction


def _patched_add_instruction(self, inst):
    si = inst.sync_info
    if si is not None and si.on_wait is not None and len(si.on_wait) > 1:
        waits = list(si.on_wait)
        for i, w in enumerate(waits[:-1]):
            nop = mybir.InstNoOp(name=f"{inst.name}-wsplit{i}", ins=[], outs=[])
            nop.engine = inst.engine
            nop.sync_info = mybir.SyncInfo(on_wait=[w], on_update=[])
            _orig_add_instruction(self, nop)
        inst.sync_info = mybir.SyncInfo(
            on_wait=[waits[-1]], on_update=list(si.on_update or []))
    _orig_add_instruction(self, inst)


def _patched_drain_and_barrier(self, tick_clock, wait_clock):
    nc = self.nc
    probe = nc.sync.nop(nofuse=True)
    wait_clock.add_sem_waits(probe.ins, ScopedClock({None: tick_clock.global_clock}))
    si = probe.ins.sync_info
    waits = list(si.on_wait) if si is not None and si.on_wait else []
    if len(waits) > 1:
        probe.ins.sync_info = mybir.SyncInfo(on_wait=waits[:1], on_update=[])
        for w in waits[1:]:
            n2 = nc.sync.nop(nofuse=True)
            n2.ins.sync_info = mybir.SyncInfo(on_wait=[w], on_update=[])
    nc.sync.drain()
    nc.all_engine_barrier()
    assert self.sems is not None
    popped = nc._tile_sem_poison_stack.pop()
    assert popped is self._sem_poison
    nc.clear_and_free_semaphores(list(self.sems.allocated().values()))
    nc.all_engine_barrier()


tile.TileContext._add_instruction = _patched_add_instruction
tile.TileContext._drain_and_barrier = _patched_drain_and_barrier


def _shear_ap(t_ap, ncols, pitch=WIN):
    """out[p, j] = flat[p*(pitch-1) + base + OFF0 + j]: per-partition start
    shifts back one element per row, staying inside each row's window."""
    return bass.AP(tensor=t_ap.tensor, offset=t_ap.offset + OFF0,
                   ap=[[pitch - 1, 128], [1, ncols]])


_FULL_TRANSPOSE = True
_PAIR_INTERLEAVE = False
MMLOG = {}  # mybir instruction name -> human label (for trace analysis)


def _mm(nc, label, *args, **kwargs):
    r = nc.tensor.matmul(*args, **kwargs)
    MMLOG[r.ins.name] = label
    return r


# ---------------------------------------------------------------------------
# device kernel build
# ---------------------------------------------------------------------------
@functools.lru_cache(maxsize=2)
def build_module(with_bias: bool):
    nc = bass.Bass()

    hid_d = nc.dram_tensor("hid", [T, H], F32, kind="ExternalInput")
    wqkT_d = nc.dram_tensor("wqkT", [H, 2 * H], F16, kind="ExternalInput")
    wvgT_d = nc.dram_tensor("wvgT", [H, 2 * H], F16, kind="ExternalInput")
    woutT_d = nc.dram_tensor("woutT", [H, H], F16, kind="ExternalInput")
    relT_d = nc.dram_tensor("relT", [H, NB], F16, kind="ExternalInput")
    Ecq_d = nc.dram_tensor("Ecq", [NB, EW], F16, kind="ExternalInput")
    Eck_d = nc.dram_tensor("Eck", [NB, EW], F16, kind="ExternalInput")
    vmask_d = nc.dram_tensor("vmask", [T, 1], F32, kind="ExternalInput")
    if with_bias:
        # host-prepared: bqkc[p, f] = b_qk[128f+p] * (SCALE if f<6 else 1)
        bqkc_d = nc.dram_tensor("bqkc", [128, 12], F32, kind="ExternalInput")
        # rows replicated for free-dim adds
        bqkr_d = nc.dram_tensor("bqkr", [1, 2 * H], F32, kind="ExternalInput")
        bvgr_d = nc.dram_tensor("bvgr", [1, 2 * H], F32, kind="ExternalInput")
        boutr_d = nc.dram_tensor("boutr", [1, H], F32, kind="ExternalInput")
    out_d = nc.dram_tensor("out", [T, H], F32, kind="ExternalOutput")

    with tile.TileContext(nc) as tc, contextlib.ExitStack() as ctx:
        persist = ctx.enter_context(tc.tile_pool(name="persist", bufs=1))
        stats = ctx.enter_context(tc.tile_pool(name="stats", bufs=4))

        # --- constants ---
        ident16 = persist.tile([128, 128], F16, tag="id16")
        make_identity(nc, ident16)
        eps_t = persist.tile([128, 1], F32, tag="eps")
        nc.vector.memset(eps_t, EPS)
        negc_t = persist.tile([128, 1], F32, tag="negc")
        nc.vector.memset(negc_t, -CSHIFT)

        # --- load weights / tables ---
        # trans pool holds tables only needed through phase 2; closed before
        # the attention pools open so its SBUF is reclaimed.
        trans_stack = contextlib.ExitStack()
        trans = trans_stack.enter_context(tc.tile_pool(name="trans", bufs=1))
        wqkT = persist.tile([128, 6, 2 * H], F16, tag="wqkT")
        wvgT = persist.tile([128, 6, 2 * H], F16, tag="wvgT")
        woutT = persist.tile([128, 6, H], F16, tag="woutT")
        relT = trans.tile([128, 6, NB], F16, tag="relT")
        for c in range(6):
            nc.sync.dma_start(out=wqkT[:, c, :], in_=wqkT_d[128 * c:128 * c + 128, :])
            nc.sync.dma_start(out=relT[:, c, :], in_=relT_d[128 * c:128 * c + 128, :])
        Ecq = trans.tile([NB, EW], F16, tag="Ecq")
        Eck = trans.tile([NB, EW], F16, tag="Eck")
        nc.sync.dma_start(out=Ecq[:], in_=Ecq_d[:])
        nc.sync.dma_start(out=Eck[:], in_=Eck_d[:])
        for c in range(6):
            nc.sync.dma_start(out=wvgT[:, c, :], in_=wvgT_d[128 * c:128 * c + 128, :])
        vmask16 = trans.tile([128, 8], F32, tag="vm")
        nc.sync.dma_start(
            out=vmask16[:],
            in_=vmask_d[:].rearrange("(t p) one -> p (t one)", p=128))
        if with_bias:
            bqkc = persist.tile([128, 12], F32, tag="bqkc")
            nc.sync.dma_start(out=bqkc[:], in_=bqkc_d[:])
            bqkr = persist.tile([64, 2 * H], F32, tag="bqkr")
            nc.sync.dma_start(
                out=bqkr[:],
                in_=bass.AP(tensor=bqkr_d, offset=0, ap=[[0, 64], [1, 2 * H]]))
            bvgr = persist.tile([128, 2 * H], F32, tag="bvgr")
            nc.sync.dma_start(
                out=bvgr[:],
                in_=bass.AP(tensor=bvgr_d, offset=0, ap=[[0, 128], [1, 2 * H]]))
            boutr = persist.tile([128, H], F32, tag="boutr")
            nc.sync.dma_start(
                out=boutr[:],
                in_=bass.AP(tensor=boutr_d, offset=0, ap=[[0, 128], [1, H]]))
        for c in range(6):
            nc.sync.dma_start(out=woutT[:, c, :], in_=woutT_d[128 * c:128 * c + 128, :])

        def layernorm_to(out16, xin, tag):
            st = stats.tile([128, 3, 6], F32, tag="bnst")
            for sg in range(3):
                nc.vector.bn_stats(out=st[:, sg, :], in_=xin[:, 256 * sg:256 * sg + 256])
            mv = stats.tile([128, 2], F32, tag="bnmv")
            nc.vector.bn_aggr(out=mv[:], in_=st[:])
            rstd = stats.tile([128, 1], F32, tag="rstd")
            nc.scalar.activation(out=rstd[:], in_=mv[:, 1:2], func=AF.Sqrt,
                                 bias=eps_t[:], scale=1.0)
            nc.vector.reciprocal(out=rstd[:], in_=rstd[:])
            nc.vector.scalar_tensor_tensor(
                out=out16, in0=xin, scalar=mv[:, 0:1],
                in1=rstd[:].to_broadcast((128, H)),
                op0=OP.subtract, op1=OP.mult)

        # --- pos projection + M matrices (PE warms up on these) ---
        posp = trans.tile([64, 2 * H], F16, tag="posp")
        Mh = persist.tile([128, 6, EW], F16, tag="Mh")
        Mq = persist.tile([128, 6, EW], F16, tag="Mq")
        qk16 = persist.tile([128, 12, T], F16, tag="qk16")
        g16 = persist.tile([128, 8, H], F16, tag="g16")
        va16 = persist.tile([128, 8, NH * 65], F16, tag="va16")
        hT = persist.tile([128, 6, T], F16, tag="hT")
        ln2T = hT  # reused after QK/VG consume hT
        ctx16 = persist.tile([128, 8, H], F16, tag="ctx16")

        with tc.tile_pool(name="ph2ps", bufs=4, space="PSUM") as ph2ps, \
             tc.tile_pool(name="ph12", bufs=2) as ph12:
            # pos projection (only needs relT + wqkT)
            for fc in range(3):
                ps = ph2ps.tile([128, 512], F32, tag="ps2")
                for c in range(6):
                    nc.tensor.matmul(
                        ps[:NB, :], relT[:, c, :], wqkT[:, c, 512 * fc:512 * fc + 512],
                        start=(c == 0), stop=(c == 5))
                if fc == 0:
                    segs = [(0, 512, SCALE)]
                elif fc == 1:
                    segs = [(0, 256, SCALE), (256, 512, 1.0)]
                else:
                    segs = [(0, 512, 1.0)]
                for (a, b_, sc) in segs:
                    if with_bias:
                        nc.vector.scalar_tensor_tensor(
                            out=posp[:NB, 512 * fc + a:512 * fc + b_],
                            in0=ps[:NB, a:b_], scalar=float(sc),
                            in1=bqkr[:NB, 512 * fc + a:512 * fc + b_],
                            op0=OP.mult, op1=OP.add)
                    else:
                        nc.vector.tensor_scalar_mul(
                            out=posp[:NB, 512 * fc + a:512 * fc + b_],
                            in0=ps[:NB, a:b_], scalar1=float(sc))
            # M matrices (per head pair; odd head in partitions 64-127).
            # Only the first 1024 columns of the EW-wide tables are ever read.
            for p in range(6):
                for half in range(2):
                    hh = 2 * p + half
                    r0 = 64 * half
                    for ec in range(2):
                        ps = ph2ps.tile([128, 512], F32, tag="ps2")
                        nc.tensor.matmul(
                            ps[r0:r0 + 64, :],
                            posp[:NB, H + 64 * hh:H + 64 * hh + 64],
                            Ecq[:, 512 * ec:512 * ec + 512],
                            start=True, stop=True, tile_position=(0, r0))
                        nc.scalar.activation(
                            out=Mh[r0:r0 + 64, p, 512 * ec:512 * ec + 512],
                            in_=ps[r0:r0 + 64, :], func=AF.Copy)
                        ps2 = ph2ps.tile([128, 512], F32, tag="ps2")
                        nc.tensor.matmul(
                            ps2[r0:r0 + 64, :],
                            posp[:NB, 64 * hh:64 * hh + 64],
                            Eck[:, 512 * ec:512 * ec + 512],
                            start=True, stop=True, tile_position=(0, r0))
                        nc.vector.tensor_copy(
                            out=Mq[r0:r0 + 64, p, 512 * ec:512 * ec + 512],
                            in_=ps2[r0:r0 + 64, :])

            # --- phase 1: LN1 per tile -> h16 -> xbar-transpose into hT ---
            for t in range(8):
                xt = ph12.tile([128, H], F32, tag="x")
                nc.sync.dma_start(out=xt[:], in_=hid_d[128 * t:128 * t + 128, :])
                h16 = ph12.tile([128, H], F16, tag="h16")
                layernorm_to(h16[:], xt[:], f"ln1_{t}")
                nc.sync.dma_start(out=hT[:, :, 128 * t:128 * t + 128],
                                  in_=h16[:], transpose=True)

            # --- phase 2: projections ---
            def vg_tile(t):
                vg_t = ph12.tile([128, 2 * H], F16, tag="vg")
                for fc in range(3):
                    ps = ph2ps.tile([128, 512], F32, tag="ps2")
                    for c in range(6):
                        nc.tensor.matmul(
                            ps[:], hT[:, c, 128 * t:128 * t + 128],
                            wvgT[:, c, 512 * fc:512 * fc + 512],
                            start=(c == 0), stop=(c == 5))
                    if with_bias:
                        nc.vector.scalar_tensor_tensor(
                            out=vg_t[:, 512 * fc:512 * fc + 512], in0=ps[:], scalar=1.0,
                            in1=bvgr[:, 512 * fc:512 * fc + 512],
                            op0=OP.mult, op1=OP.add)
                    else:
                        nc.vector.tensor_copy(
                            out=vg_t[:, 512 * fc:512 * fc + 512], in_=ps[:])
                nc.scalar.activation(out=g16[:, t, :], in_=vg_t[:, H:2 * H], func=AF.Gelu)
                for hh in range(NH):
                    nc.vector.tensor_scalar_mul(
                        out=va16[:, t, 65 * hh:65 * hh + 64],
                        in0=vg_t[:, 64 * hh:64 * hh + 64],
                        scalar1=vmask16[:, t:t + 1])
                vav = va16[:, t, :].rearrange("p (h c) -> p h c", h=NH)
                nc.vector.tensor_copy(
                    out=vav[:, :, 64],
                    in_=vmask16[:, t:t + 1].to_broadcast((128, NH)))

            def qk_half(nh):
                for f in range(12):
                    ps = ph2ps.tile([128, 512], F32, tag="ps2")
                    for c in range(6):
                        nc.tensor.matmul(
                            ps[:], wqkT[:, c, 128 * f:128 * f + 128],
                            hT[:, c, 512 * nh:512 * nh + 512],
                            start=(c == 0), stop=(c == 5))
                    if with_bias:
                        nc.scalar.activation(
                            out=qk16[:, f, 512 * nh:512 * nh + 512], in_=ps[:],
                            func=AF.Identity, bias=bqkc[:, f:f + 1],
                            scale=SCALE if f < 6 else 1.0)
                    else:
                        nc.scalar.activation(
                            out=qk16[:, f, 512 * nh:512 * nh + 512], in_=ps[:],
                            func=AF.Copy, bias=0.0,
                            scale=SCALE if f < 6 else 1.0)

            for t in range(4):
                vg_tile(t)
            qk_half(0)
            for t in range(4, 8):
                vg_tile(t)
            qk_half(1)
        trans_stack.close()

        # --- phase 3 attention + phase 4 epilogue, per batch ---
        with tc.tile_pool(name="wps", bufs=1, space="PSUM") as wps, \
             tc.tile_pool(name="scps", bufs=2, space="PSUM") as scps, \
             tc.tile_pool(name="pvps", bufs=2, space="PSUM") as pvps, \
             tc.tile_pool(name="shear", bufs=2) as shp, \
             tc.tile_pool(name="etp", bufs=4) as etp, \
             tc.tile_pool(name="ph4", bufs=2) as ph4:
            def emit_expansions(b, hh):
                tok0 = 512 * b
                p, half = hh // 2, hh % 2
                r0 = 64 * half
                cqT = shp.tile([128, 4, 4, 128], F16, tag="cqT", bufs=3)   # [kl, t, u, q]
                cqsh = shp.tile([128, 4, 512], F16, tag="cqsh")    # [q, qt, k]
                cksh = shp.tile([128, 4, 512], F16, tag="cksh", bufs=3)    # [kl, kt, q]
                for t in range(4):
                    ws = 384 - 128 * t
                    lq = qk16[r0:r0 + 64, p, tok0 + 128 * t:tok0 + 128 * t + 128]
                    lk = qk16[r0:r0 + 64, 6 + p, tok0 + 128 * t:tok0 + 128 * t + 128]
                    wq = shp.tile([128, WIN], F16, tag="wcq")
                    wk = shp.tile([128, WIN], F16, tag="wck")
                    pa = wps.tile([128, 512], F32, tag="wpsa", bufs=2)
                    pbp = wps.tile([128, 256], F32, tag="wpsb", bufs=1)
                    _mm(nc, f'exp-qa-{t}', pa[:], lq, Mh[r0:r0 + 64, p, ws:ws + 512],
                        start=True, stop=True)
                    _mm(nc, f'exp-qb-{t}', pbp[:, 0:128], lq,
                        Mh[r0:r0 + 64, p, ws + 512:ws + 640], start=True, stop=True)
                    nc.scalar.activation(out=wq[:, :512], in_=pa[:], func=AF.Copy)
                    nc.scalar.activation(out=wq[:, 512:640], in_=pbp[:, 0:128], func=AF.Copy)
                    pa2 = wps.tile([128, 512], F32, tag="wpsa2", bufs=2)
                    _mm(nc, f'exp-ka-{t}', pa2[:], lk, Mq[r0:r0 + 64, p, ws:ws + 512],
                        start=True, stop=True)
                    _mm(nc, f'exp-kb-{t}', pbp[:, 128:256], lk,
                        Mq[r0:r0 + 64, p, ws + 512:ws + 640], start=True, stop=True)
                    nc.vector.tensor_copy(out=wk[:, :512], in_=pa2[:])
                    nc.vector.tensor_copy(out=wk[:, 512:640], in_=pbp[:, 128:256])
                    # plain shears on the gpsimd swdge queue
                    nc.gpsimd.dma_start(out=cqsh[:, t, :], in_=_shear_ap(wq[:], 512))
                    nc.gpsimd.dma_start(out=cksh[:, t, :], in_=_shear_ap(wk[:], 512))
                # one xbar-transpose for the whole head: [q,(t,k)] -> [kl,(t,u),q]
                nc.sync.dma_start(out=cqT[:], in_=cqsh[:], transpose=True)
                return cqT, cksh

            def emit_scores_pv(b, hh, cqT, cksh):
                tok0 = 512 * b
                p, half = hh // 2, hh % 2
                r0 = 64 * half
                cpsT = pvps.tile([80, 512], F32, tag="cpsT", bufs=1)
                va_h = va16[:, :, 65 * hh:65 * hh + 65]
                ets = []

                def pv_mms(u):
                    for t in range(4):
                        _mm(nc, f'pv-{u}-{t}',
                            cpsT[:65, 128 * t:128 * t + 128],
                            va_h[:, 4 * b + u, :],
                            ets[u][:, 128 * t:128 * t + 128],
                            start=(u == 0 and t == 0), stop=(u == 3 and t == 3))

                for u in range(4):
                    sc = scps.tile([128, 512], F32, tag="sc")
                    _mm(nc, f'sc-cc-{u}',
                        sc[:],
                        qk16[r0:r0 + 64, 6 + p, tok0 + 128 * u:tok0 + 128 * u + 128],
                        qk16[r0:r0 + 64, p, tok0:tok0 + 512],
                        start=True, stop=False)
                    _mm(nc, f'sc-ck-{u}', sc[:], ident16[:], cksh[:, u, :],
                                     start=False, stop=False)
                    _mm(nc, f'sc-cq-{u}', sc[:], ident16[:], cqT[:, :, u, :],
                                     start=False, stop=True)
                    e_u = etp.tile([128, 512], F16, tag="et")
                    nc.scalar.activation(out=e_u[:], in_=sc[:], func=AF.Exp,
                                         bias=negc_t[:], scale=1.0)
                    ets.append(e_u)
                    pv_mms(u)
                # -- evict ctx^T; transpose + divide deferred one cycle --
                cps16 = shp.tile([80, 512], F16, tag="cps16", bufs=3)
                if emit_scores_pv.n < 3:
                    nc.vector.memset(cps16[64:80, :], 0.0)
                emit_scores_pv.n += 1
                nc.vector.tensor_copy(out=cps16[:65, :], in_=cpsT[:65, :])
                return cps16

            def emit_ctx_finish(b, hh, cps16):
                ctxq = shp.tile([128, 4, 80], F16, tag="ctxq")
                nc.sync.dma_start(out=ctxq[:], in_=cps16[:80, :], transpose=True)
                rec4 = stats.tile([128, 4], F32, tag="rec")
                nc.vector.reciprocal(out=rec4[:], in_=ctxq[:, :, 64])
                for t in range(4):
                    nc.vector.tensor_scalar_mul(
                        out=ctx16[:, 4 * b + t, 64 * hh:64 * hh + 64],
                        in0=ctxq[:, t, 0:64], scalar1=rec4[:, t:t + 1])

            emit_scores_pv.n = 0

            def emit_p4(b):
                # --- phase 4 for this batch: gate, LN2, transpose, out proj ---
                for g in range(4 * b, 4 * b + 4):
                    cg = ph4.tile([128, H], F16, tag="cg")
                    nc.vector.tensor_mul(cg[:], ctx16[:, g, :], g16[:, g, :])
                    ln2 = ph4.tile([128, H], F16, tag="ln2")
                    layernorm_to(ln2[:], cg[:], f"ln2_{g}")
                    nc.sync.dma_start(out=ln2T[:, :, 128 * g:128 * g + 128],
                                      in_=ln2[:], transpose=True)
                for g in range(4 * b, 4 * b + 4):
                    ot = ph4.tile([128, H], F32, tag="ot")  # noqa: indent kept
                    for fc, (f0, fw) in enumerate([(0, 512), (512, 256)]):
                        ps = scps.tile([128, 512], F32, tag="sc")
                        for c in range(6):
                            nc.tensor.matmul(
                                ps[:, :fw], ln2T[:, c, 128 * g:128 * g + 128],
                                woutT[:, c, f0:f0 + fw],
                                start=(c == 0), stop=(c == 5))
                        if with_bias:
                            nc.vector.scalar_tensor_tensor(
                                out=ot[:, f0:f0 + fw], in0=ps[:, :fw], scalar=1.0,
                                in1=boutr[:, f0:f0 + fw], op0=OP.mult, op1=OP.add)
                        else:
                            nc.vector.tensor_copy(out=ot[:, f0:f0 + fw], in_=ps[:, :fw])
                    nc.sync.dma_start(out=out_d[128 * g:128 * g + 128, :], in_=ot[:])

            # software-pipelined driver: head i+1's expansions are emitted
            # before head i's score/PV matmuls so the shear+transpose DMA
            # latency hides behind real PE work.
            from collections import deque
            work = [(b, hh) for b in range(BL) for hh in range(NH)]
            pend = deque()
            fin = deque()
            def drain_fin():
                (fb_, fh_), fcps = fin.popleft()
                emit_ctx_finish(fb_, fh_, fcps)
                if fh_ == NH - 1:
                    emit_p4(fb_)
            def drain_one():
                (pb_, ph_), (pcqT, pcksh) = pend.popleft()
                cps = emit_scores_pv(pb_, ph_, pcqT, pcksh)
                fin.append(((pb_, ph_), cps))
                if len(fin) > 1:
                    drain_fin()
            for w in work:
                tiles = emit_expansions(*w)
                pend.append((w, tiles))
                if len(pend) > 2:
                    drain_one()
            while pend:
                drain_one()
            while fin:
                drain_fin()

    return nc


# ---------------------------------------------------------------------------
# host side
# ---------------------------------------------------------------------------
def _host_prep(position_indices, attention_mask):
    pi = np.asarray(position_indices)
    gvec = np.empty(1023, np.int64)
    gvec[511:] = pi[:, 0]
    gvec[:512] = pi[0, ::-1]
    d = np.arange(S)[:, None] - np.arange(S)[None, :]
    assert np.array_equal(gvec[d + 511], pi), "position_indices not Toeplitz"
    e = np.arange(1023)
    # table col c maps to window col j = c - ws; one zero col prepended so the
    # t=3 window slice starts at col 0 (cq: c = 512 - delta, ck: c = 512 + delta)
    E_cq = np.zeros((NB, EW), np.float16)
    E_ck = np.zeros((NB, EW), np.float16)
    E_cq[:, 1:1024] = (np.arange(NB)[:, None] == gvec[1022 - e][None, :])
    E_ck[:, 1:1024] = (np.arange(NB)[:, None] == gvec[e][None, :])
    am = np.asarray(attention_mask).reshape(B, S)
    vmask = (~am).astype(np.float32)
    return E_cq, E_ck, vmask


def kernel(hidden_states, relative_embedding, w_qk, b_qk, w_vg, b_vg,
           w_out, b_out, attention_mask, position_indices):
    from concourse.bass_utils import run_bass_kernel_spmd

    hidden_states = np.asarray(hidden_states, dtype=np.float32)
    relative_embedding = np.asarray(relative_embedding, dtype=np.float32)
    w_qk = np.asarray(w_qk, dtype=np.float32)
    w_vg = np.asarray(w_vg, dtype=np.float32)
    w_out = np.asarray(w_out, dtype=np.float32)
    b_qk = np.asarray(b_qk, dtype=np.float32)
    b_vg = np.asarray(b_vg, dtype=np.float32)
    b_out = np.asarray(b_out, dtype=np.float32)

    with_bias = bool(np.any(b_qk) or np.any(b_vg) or np.any(b_out))
    E_cq, E_ck, vmask = _host_prep(position_indices, attention_mask)

    nc = build_module(with_bias)
    common = dict(
        wqkT=np.ascontiguousarray(w_qk.T).astype(np.float16),
        wvgT=np.ascontiguousarray(w_vg.T).astype(np.float16),
        woutT=np.ascontiguousarray(w_out.T).astype(np.float16),
        relT=np.ascontiguousarray(relative_embedding.T).astype(np.float16),
        Ecq=E_cq, Eck=E_ck)
    if with_bias:
        sc_col = np.where(np.arange(12) < 6, SCALE, 1.0).astype(np.float32)
        common["bqkc"] = np.ascontiguousarray(
            b_qk.reshape(12, 128).T * sc_col[None, :])
        sc_row = np.concatenate([np.full(H, SCALE), np.ones(H)]).astype(np.float32)
        common["bqkr"] = (b_qk * sc_row)[None, :].astype(np.float32)
        common["bvgr"] = b_vg[None, :].astype(np.float32)
        common["boutr"] = b_out[None, :].astype(np.float32)

    in_maps = []
    for core in range(NCORES):
        bsel = [BL * core + i for i in range(BL)]
        hid = np.ascontiguousarray(
            hidden_states[:, bsel, :].transpose(1, 0, 2).reshape(T, H))
        vm = np.ascontiguousarray(vmask[bsel].reshape(T, 1))
        in_maps.append(dict(common, hid=hid, vmask=vm))

    res = run_bass_kernel_spmd(nc, in_maps, list(range(NCORES)))
    out = np.empty((S, B, H), np.float32)
    for core in range(NCORES):
        o = res.results[core]["out"].reshape(BL, S, H)
        for i in range(BL):
            out[:, BL * core + i, :] = o[i]
    return out


# revision 48
# speedup vs baseline: 1.1385x; 1.0031x over previous
"""Trainium2 Bass kernel for nn_Bert_44452911514066 (DeBERTa-style disentangled
attention BERT layer), data-parallel over batch across 8 NeuronCores.

kernel(**inputs) takes the FULL inputs (as produced by reference.setup_inputs)
and returns the FULL [S, B, H] output.

Key ideas:
  - batch-DP: 2 batches per core, weights/tables replicated.
  - the relative-position gather is Toeplitz: per (b,h), bucket values are
    expanded into "diagonal space" by matmuls (rhs = per-head M matrices built
    from a one-hot bucket expansion); the diagonal shear is applied by DMAs
    whose access pattern steps (partition+1, elem-1) over the window tiles.
  - scores are assembled transposed [k, q] in PSUM: CC matmul + identity
    matmuls of the sheared cq/ck tiles. The cq tiles (natural [q, k]) are
    sheared AND transposed in a single xbar-transpose DMA (stride 656 rows).
    No PE transposes in the attention loop -> the PE HAM stays at K=8/8.
  - softmax without max-subtraction: exp(s - 12) on ScalarE; masking and the
    denominator are folded into an augmented/masked V matrix. PV runs with V
    stationary into one [65, 512] PSUM bank; the ctx^T result is transposed
    back by xbar DMAs and divided on DVE.
  - LN1/LN2 transposes are xbar DMAs per 128-token tile (no DRAM roundtrip).
  - fp16 matmul inputs everywhere (full PE rate), fp32 accumulation.
"""
import sys
sys.path.insert(0, "/opt/trn_rl_repo")
import math
import functools
import contextlib
import numpy as np

import concourse.bass as bass
import concourse.tile as tile
from concourse import mybir
from concourse.masks import make_identity

H, NH, HD, S, B = 768, 12, 64, 512, 16
NCORES = 8
BL = B // NCORES          # batches per core
T = BL * S                # tokens per core
SCALE = 1.0 / math.sqrt(3 * HD)
EPS = 1e-7
NB = 63                   # relative buckets
WIN = 657                 # window elems per row; stride WIN-1=656 is 32B-aligned
OFF0 = 128                # shear: window col j = OFF0 + k - q
EW = 1032                 # padded E-table width (zero col + 1024 data + pad)
CSHIFT = 12.0             # exp shift
F16 = mybir.dt.float16
F32 = mybir.dt.float32
AF = mybir.ActivationFunctionType
OP = mybir.AluOpType

# ---------------------------------------------------------------------------
# walrus workaround: this container's walrus accepts at most ONE sync wait per
# instruction; split extra waits onto single-wait NoOps.
# ---------------------------------------------------------------------------
from concourse.vector_clock import ScopedClock

_orig_add_instruction = tile.TileContext._add_instruction


def _patched_add_instruction(self, inst):
    si = inst.sync_info
    if si is not None and si.on_wait is not None and len(si.on_wait) > 1:
        waits = list(si.on_wait)
        for i, w in enumerate(waits[:-1]):
            nop = mybir.InstNoOp(name=f"{inst.name}-wsplit{i}", ins=[], outs=[])
            nop.engine = inst.engine
            nop.sync_info = mybir.SyncInfo(on_wait=[w], on_update=[])
            _orig_add_instruction(self, nop)
        inst.sync_info = mybir.SyncInfo(
            on_wait=[waits[-1]], on_update=list(si.on_update or []))
    _orig_add_instruction(self, inst)


def _patched_drain_and_barrier(self, tick_clock, wait_clock):
    nc = self.nc
    probe = nc.sync.nop(nofuse=True)
    wait_clock.add_sem_waits(probe.ins, ScopedClock({None: tick_clock.global_clock}))
    si = probe.ins.sync_info
    waits = list(si.on_wait) if si is not None and si.on_wait else []
    if len(waits) > 1:
        probe.ins.sync_info = mybir.SyncInfo(on_wait=waits[:1], on_update=[])
        for w in waits[1:]:
            n2 = nc.sync.nop(nofuse=True)
            n2.ins.sync_info = mybir.SyncInfo(on_wait=[w], on_update=[])
    nc.sync.drain()
    nc.all_engine_barrier()
    assert self.sems is not None
    popped = nc._tile_sem_poison_stack.pop()
    assert popped is self._sem_poison
    nc.clear_and_free_semaphores(list(self.sems.allocated().values()))
    nc.all_engine_barrier()


tile.TileContext._add_instruction = _patched_add_instruction
tile.TileContext._drain_and_barrier = _patched_drain_and_barrier


def _shear_ap(t_ap, ncols, pitch=WIN):
    """out[p, j] = flat[p*(pitch-1) + base + OFF0 + j]: per-partition start
    shifts back one element per row, staying inside each row's window."""
    return bass.AP(tensor=t_ap.tensor, offset=t_ap.offset + OFF0,
                   ap=[[pitch - 1, 128], [1, ncols]])


_FULL_TRANSPOSE = True
_PAIR_INTERLEAVE = False
MMLOG = {}  # mybir instruction name -> human label (for trace analysis)


def _mm(nc, label, *args, **kwargs):
    r = nc.tensor.matmul(*args, **kwargs)
    MMLOG[r.ins.name] = label
    return r


# ---------------------------------------------------------------------------
# device kernel build
# ---------------------------------------------------------------------------
@functools.lru_cache(maxsize=2)
def build_module(with_bias: bool):
    nc = bass.Bass()

    hid_d = nc.dram_tensor("hid", [T, H], F32, kind="ExternalInput")
    wqkT_d = nc.dram_tensor("wqkT", [H, 2 * H], F16, kind="ExternalInput")
    wvgT_d = nc.dram_tensor("wvgT", [H, 2 * H], F16, kind="ExternalInput")
    woutT_d = nc.dram_tensor("woutT", [H, H], F16, kind="ExternalInput")
    relT_d = nc.dram_tensor("relT", [H, NB], F16, kind="ExternalInput")
    Ecq_d = nc.dram_tensor("Ecq", [NB, EW], F16, kind="ExternalInput")
    Eck_d = nc.dram_tensor("Eck", [NB, EW], F16, kind="ExternalInput")
    vmask_d = nc.dram_tensor("vmask", [T, 1], F32, kind="ExternalInput")
    if with_bias:
        # host-prepared: bqkc[p, f] = b_qk[128f+p] * (SCALE if f<6 else 1)
        bqkc_d = nc.dram_tensor("bqkc", [128, 12], F32, kind="ExternalInput")
        # rows replicated for free-dim adds
        bqkr_d = nc.dram_tensor("bqkr", [1, 2 * H], F32, kind="ExternalInput")
        bvgr_d = nc.dram_tensor("bvgr", [1, 2 * H], F32, kind="ExternalInput")
        boutr_d = nc.dram_tensor("boutr", [1, H], F32, kind="ExternalInput")
    out_d = nc.dram_tensor("out", [T, H], F32, kind="ExternalOutput")

    with tile.TileContext(nc) as tc, contextlib.ExitStack() as ctx:
        persist = ctx.enter_context(tc.tile_pool(name="persist", bufs=1))
        stats = ctx.enter_context(tc.tile_pool(name="stats", bufs=4))

        # --- constants ---
        ident16 = persist.tile([128, 128], F16, tag="id16")
        make_identity(nc, ident16)
        eps_t = persist.tile([128, 1], F32, tag="eps")
        nc.vector.memset(eps_t, EPS)
        negc_t = persist.tile([128, 1], F32, tag="negc")
        nc.vector.memset(negc_t, -CSHIFT)

        # --- load weights / tables ---
        # trans pool holds tables only needed through phase 2; closed before
        # the attention pools open so its SBUF is reclaimed.
        trans_stack = contextlib.ExitStack()
        trans = trans_stack.enter_context(tc.tile_pool(name="trans", bufs=1))
        wqkT = persist.tile([128, 6, 2 * H], F16, tag="wqkT")
        wvgT = persist.tile([128, 6, 2 * H], F16, tag="wvgT")
        woutT = persist.tile([128, 6, H], F16, tag="woutT")
        relT = trans.tile([128, 6, NB], F16, tag="relT")
        for c in range(6):
            nc.sync.dma_start(out=wqkT[:, c, :], in_=wqkT_d[128 * c:128 * c + 128, :])
            nc.sync.dma_start(out=relT[:, c, :], in_=relT_d[128 * c:128 * c + 128, :])
        Ecq = trans.tile([NB, EW], F16, tag="Ecq")
        Eck = trans.tile([NB, EW], F16, tag="Eck")
        nc.sync.dma_start(out=Ecq[:], in_=Ecq_d[:])
        nc.sync.dma_start(out=Eck[:], in_=Eck_d[:])
        for c in range(6):
            nc.sync.dma_start(out=wvgT[:, c, :], in_=wvgT_d[128 * c:128 * c + 128, :])
        vmask16 = trans.tile([128, 8], F32, tag="vm")
        nc.sync.dma_start(
            out=vmask16[:],
            in_=vmask_d[:].rearrange("(t p) one -> p (t one)", p=128))
        if with_bias:
            bqkc = persist.tile([128, 12], F32, tag="bqkc")
            nc.sync.dma_start(out=bqkc[:], in_=bqkc_d[:])
            bqkr = persist.tile([64, 2 * H], F32, tag="bqkr")
            nc.sync.dma_start(
                out=bqkr[:],
                in_=bass.AP(tensor=bqkr_d, offset=0, ap=[[0, 64], [1, 2 * H]]))
            bvgr = persist.tile([128, 2 * H], F32, tag="bvgr")
            nc.sync.dma_start(
                out=bvgr[:],
                in_=bass.AP(tensor=bvgr_d, offset=0, ap=[[0, 128], [1, 2 * H]]))
            boutr = persist.tile([128, H], F32, tag="boutr")
            nc.sync.dma_start(
                out=boutr[:],
                in_=bass.AP(tensor=boutr_d, offset=0, ap=[[0, 128], [1, H]]))
        for c in range(6):
            nc.sync.dma_start(out=woutT[:, c, :], in_=woutT_d[128 * c:128 * c + 128, :])

        def layernorm_to(out16, xin, tag):
            st = stats.tile([128, 3, 6], F32, tag="bnst")
            for sg in range(3):
                nc.vector.bn_stats(out=st[:, sg, :], in_=xin[:, 256 * sg:256 * sg + 256])
            mv = stats.tile([128, 2], F32, tag="bnmv")
            nc.vector.bn_aggr(out=mv[:], in_=st[:])
            rstd = stats.tile([128, 1], F32, tag="rstd")
            nc.scalar.activation(out=rstd[:], in_=mv[:, 1:2], func=AF.Sqrt,
                                 bias=eps_t[:], scale=1.0)
            nc.vector.reciprocal(out=rstd[:], in_=rstd[:])
            nc.vector.scalar_tensor_tensor(
                out=out16, in0=xin, scalar=mv[:, 0:1],
                in1=rstd[:].to_broadcast((128, H)),
                op0=OP.subtract, op1=OP.mult)

        # --- pos projection + M matrices (PE warms up on these) ---
        posp = trans.tile([64, 2 * H], F16, tag="posp")
        Mh = persist.tile([128, 6, EW], F16, tag="Mh")
        Mq = persist.tile([128, 6, EW], F16, tag="Mq")
        qk16 = persist.tile([128, 12, T], F16, tag="qk16")
        g16 = persist.tile([128, 8, H], F16, tag="g16")
        va16 = persist.tile([128, 8, NH * 65], F16, tag="va16")
        hT = persist.tile([128, 6, T], F16, tag="hT")
        ln2T = hT  # reused after QK/VG consume hT
        ctx16 = persist.tile([128, 8, H], F16, tag="ctx16")

        with tc.tile_pool(name="ph2ps", bufs=4, space="PSUM") as ph2ps, \
             tc.tile_pool(name="ph12", bufs=2) as ph12:
            # pos projection (only needs relT + wqkT)
            for fc in range(3):
                ps = ph2ps.tile([128, 512], F32, tag="ps2")
                for c in range(6):
                    nc.tensor.matmul(
                        ps[:NB, :], relT[:, c, :], wqkT[:, c, 512 * fc:512 * fc + 512],
                        start=(c == 0), stop=(c == 5))
                if fc == 0:
                    segs = [(0, 512, SCALE)]
                elif fc == 1:
                    segs = [(0, 256, SCALE), (256, 512, 1.0)]
                else:
                    segs = [(0, 512, 1.0)]
                for (a, b_, sc) in segs:
                    if with_bias:
                        nc.vector.scalar_tensor_tensor(
                            out=posp[:NB, 512 * fc + a:512 * fc + b_],
                            in0=ps[:NB, a:b_], scalar=float(sc),
                            in1=bqkr[:NB, 512 * fc + a:512 * fc + b_],
                            op0=OP.mult, op1=OP.add)
                    else:
                        nc.vector.tensor_scalar_mul(
                            out=posp[:NB, 512 * fc + a:512 * fc + b_],
                            in0=ps[:NB, a:b_], scalar1=float(sc))
            # M matrices (per head pair; odd head in partitions 64-127).
            # Only the first 1024 columns of the EW-wide tables are ever read.
            for p in range(6):
                for half in range(2):
                    hh = 2 * p + half
                    r0 = 64 * half
                    for ec in range(2):
                        ps = ph2ps.tile([128, 512], F32, tag="ps2")
                        nc.tensor.matmul(
                            ps[r0:r0 + 64, :],
                            posp[:NB, H + 64 * hh:H + 64 * hh + 64],
                            Ecq[:, 512 * ec:512 * ec + 512],
                            start=True, stop=True, tile_position=(0, r0))
                        nc.scalar.activation(
                            out=Mh[r0:r0 + 64, p, 512 * ec:512 * ec + 512],
                            in_=ps[r0:r0 + 64, :], func=AF.Copy)
                        ps2 = ph2ps.tile([128, 512], F32, tag="ps2")
                        nc.tensor.matmul(
                            ps2[r0:r0 + 64, :],
                            posp[:NB, 64 * hh:64 * hh + 64],
                            Eck[:, 512 * ec:512 * ec + 512],
                            start=True, stop=True, tile_position=(0, r0))
                        nc.vector.tensor_copy(
                            out=Mq[r0:r0 + 64, p, 512 * ec:512 * ec + 512],
                            in_=ps2[r0:r0 + 64, :])

            # --- phase 1: LN1 per tile -> h16 -> xbar-transpose into hT ---
            for t in range(8):
                xt = ph12.tile([128, H], F32, tag="x")
                nc.sync.dma_start(out=xt[:], in_=hid_d[128 * t:128 * t + 128, :])
                h16 = ph12.tile([128, H], F16, tag="h16")
                layernorm_to(h16[:], xt[:], f"ln1_{t}")
                nc.sync.dma_start(out=hT[:, :, 128 * t:128 * t + 128],
                                  in_=h16[:], transpose=True)

            # --- phase 2: projections ---
            def vg_tile(t):
                vg_t = ph12.tile([128, 2 * H], F16, tag="vg")
                for fc in range(3):
                    ps = ph2ps.tile([128, 512], F32, tag="ps2")
                    for c in range(6):
                        nc.tensor.matmul(
                            ps[:], hT[:, c, 128 * t:128 * t + 128],
                            wvgT[:, c, 512 * fc:512 * fc + 512],
                            start=(c == 0), stop=(c == 5))
                    if with_bias:
                        nc.vector.scalar_tensor_tensor(
                            out=vg_t[:, 512 * fc:512 * fc + 512], in0=ps[:], scalar=1.0,
                            in1=bvgr[:, 512 * fc:512 * fc + 512],
                            op0=OP.mult, op1=OP.add)
                    else:
                        nc.vector.tensor_copy(
                            out=vg_t[:, 512 * fc:512 * fc + 512], in_=ps[:])
                nc.scalar.activation(out=g16[:, t, :], in_=vg_t[:, H:2 * H], func=AF.Gelu)
                for hh in range(NH):
                    nc.vector.tensor_scalar_mul(
                        out=va16[:, t, 65 * hh:65 * hh + 64],
                        in0=vg_t[:, 64 * hh:64 * hh + 64],
                        scalar1=vmask16[:, t:t + 1])
                vav = va16[:, t, :].rearrange("p (h c) -> p h c", h=NH)
                nc.vector.tensor_copy(
                    out=vav[:, :, 64],
                    in_=vmask16[:, t:t + 1].to_broadcast((128, NH)))

            def qk_half(nh):
                for f in range(12):
                    ps = ph2ps.tile([128, 512], F32, tag="ps2")
                    for c in range(6):
                        nc.tensor.matmul(
                            ps[:], wqkT[:, c, 128 * f:128 * f + 128],
                            hT[:, c, 512 * nh:512 * nh + 512],
                            start=(c == 0), stop=(c == 5))
                    if with_bias:
                        nc.scalar.activation(
                            out=qk16[:, f, 512 * nh:512 * nh + 512], in_=ps[:],
                            func=AF.Identity, bias=bqkc[:, f:f + 1],
                            scale=SCALE if f < 6 else 1.0)
                    else:
                        nc.scalar.activation(
                            out=qk16[:, f, 512 * nh:512 * nh + 512], in_=ps[:],
                            func=AF.Copy, bias=0.0,
                            scale=SCALE if f < 6 else 1.0)

            for t in range(4):
                vg_tile(t)
            qk_half(0)
            for t in range(4, 8):
                vg_tile(t)
            qk_half(1)
        trans_stack.close()

        # --- phase 3 attention + phase 4 epilogue, per batch ---
        with tc.tile_pool(name="wps", bufs=1, space="PSUM") as wps, \
             tc.tile_pool(name="scps", bufs=2, space="PSUM") as scps, \
             tc.tile_pool(name="pvps", bufs=2, space="PSUM") as pvps, \
             tc.tile_pool(name="shear", bufs=2) as shp, \
             tc.tile_pool(name="etp", bufs=4) as etp, \
             tc.tile_pool(name="ph4", bufs=2) as ph4:
            def emit_expansions(b, hh):
                tok0 = 512 * b
                p, half = hh // 2, hh % 2
                r0 = 64 * half
                cqT = shp.tile([128, 4, 4, 128], F16, tag="cqT", bufs=3)   # [kl, t, u, q]
                cqsh = shp.tile([128, 4, 512], F16, tag="cqsh")    # [q, qt, k]
                cksh = shp.tile([128, 4, 512], F16, tag="cksh", bufs=3)    # [kl, kt, q]
                for t in range(4):
                    ws = 384 - 128 * t
                    lq = qk16[r0:r0 + 64, p, tok0 + 128 * t:tok0 + 128 * t + 128]
                    lk = qk16[r0:r0 + 64, 6 + p, tok0 + 128 * t:tok0 + 128 * t + 128]
                    wq = shp.tile([128, WIN], F16, tag="wcq", bufs=4)
                    wk = shp.tile([128, WIN], F16, tag="wck", bufs=4)
                    pa = wps.tile([128, 512], F32, tag="wpsa", bufs=2)
                    pbp = wps.tile([128, 256], F32, tag="wpsb", bufs=1)
                    _mm(nc, f'exp-qa-{t}', pa[:], lq, Mh[r0:r0 + 64, p, ws:ws + 512],
                        start=True, stop=True)
                    _mm(nc, f'exp-qb-{t}', pbp[:, 0:128], lq,
                        Mh[r0:r0 + 64, p, ws + 512:ws + 640], start=True, stop=True)
                    nc.scalar.activation(out=wq[:, :512], in_=pa[:], func=AF.Copy)
                    nc.scalar.activation(out=wq[:, 512:640], in_=pbp[:, 0:128], func=AF.Copy)
                    pa2 = wps.tile([128, 512], F32, tag="wpsa2", bufs=2)
                    _mm(nc, f'exp-ka-{t}', pa2[:], lk, Mq[r0:r0 + 64, p, ws:ws + 512],
                        start=True, stop=True)
                    _mm(nc, f'exp-kb-{t}', pbp[:, 128:256], lk,
                        Mq[r0:r0 + 64, p, ws + 512:ws + 640], start=True, stop=True)
                    nc.vector.tensor_copy(out=wk[:, :512], in_=pa2[:])
                    nc.vector.tensor_copy(out=wk[:, 512:640], in_=pbp[:, 128:256])
                    # plain shears on the gpsimd swdge queue
                    nc.gpsimd.dma_start(out=cqsh[:, t, :], in_=_shear_ap(wq[:], 512))
                    nc.gpsimd.dma_start(out=cksh[:, t, :], in_=_shear_ap(wk[:], 512))
                # one xbar-transpose for the whole head: [q,(t,k)] -> [kl,(t,u),q]
                nc.sync.dma_start(out=cqT[:], in_=cqsh[:], transpose=True)
                return cqT, cksh

            def emit_scores_pv(b, hh, cqT, cksh):
                tok0 = 512 * b
                p, half = hh // 2, hh % 2
                r0 = 64 * half
                cpsT = pvps.tile([80, 512], F32, tag="cpsT", bufs=1)
                va_h = va16[:, :, 65 * hh:65 * hh + 65]
                ets = []

                def pv_mms(u):
                    for t in range(4):
                        _mm(nc, f'pv-{u}-{t}',
                            cpsT[:65, 128 * t:128 * t + 128],
                            va_h[:, 4 * b + u, :],
                            ets[u][:, 128 * t:128 * t + 128],
                            start=(u == 0 and t == 0), stop=(u == 3 and t == 3))

                for u in range(4):
                    sc = scps.tile([128, 512], F32, tag="sc")
                    _mm(nc, f'sc-cc-{u}',
                        sc[:],
                        qk16[r0:r0 + 64, 6 + p, tok0 + 128 * u:tok0 + 128 * u + 128],
                        qk16[r0:r0 + 64, p, tok0:tok0 + 512],
                        start=True, stop=False)
                    _mm(nc, f'sc-ck-{u}', sc[:], ident16[:], cksh[:, u, :],
                                     start=False, stop=False)
                    _mm(nc, f'sc-cq-{u}', sc[:], ident16[:], cqT[:, :, u, :],
                                     start=False, stop=True)
                    e_u = etp.tile([128, 512], F16, tag="et")
                    nc.scalar.activation(out=e_u[:], in_=sc[:], func=AF.Exp,
                                         bias=negc_t[:], scale=1.0)
                    ets.append(e_u)
                    pv_mms(u)
                # -- evict ctx^T; transpose + divide deferred one cycle --
                cps16 = shp.tile([80, 512], F16, tag="cps16", bufs=3)
                if emit_scores_pv.n < 3:
                    nc.vector.memset(cps16[64:80, :], 0.0)
                emit_scores_pv.n += 1
                nc.vector.tensor_copy(out=cps16[:65, :], in_=cpsT[:65, :])
                return cps16

            def emit_ctx_finish(b, hh, cps16):
                ctxq = shp.tile([128, 4, 80], F16, tag="ctxq")
                nc.sync.dma_start(out=ctxq[:], in_=cps16[:80, :], transpose=True)
                rec4 = stats.tile([128, 4], F32, tag="rec")
                nc.vector.reciprocal(out=rec4[:], in_=ctxq[:, :, 64])
                for t in range(4):
                    nc.vector.tensor_scalar_mul(
                        out=ctx16[:, 4 * b + t, 64 * hh:64 * hh + 64],
                        in0=ctxq[:, t, 0:64], scalar1=rec4[:, t:t + 1])

            emit_scores_pv.n = 0

            def emit_p4(b):
                # --- phase 4 for this batch: gate, LN2, transpose, out proj ---
                for g in range(4 * b, 4 * b + 4):
                    cg = ph4.tile([128, H], F16, tag="cg")
                    nc.vector.tensor_mul(cg[:], ctx16[:, g, :], g16[:, g, :])
                    ln2 = ph4.tile([128, H], F16, tag="ln2")
                    layernorm_to(ln2[:], cg[:], f"ln2_{g}")
                    nc.sync.dma_start(out=ln2T[:, :, 128 * g:128 * g + 128],
                                      in_=ln2[:], transpose=True)
                for g in range(4 * b, 4 * b + 4):
                    ot = ph4.tile([128, H], F32, tag="ot")  # noqa: indent kept
                    for fc, (f0, fw) in enumerate([(0, 512), (512, 256)]):
                        ps = scps.tile([128, 512], F32, tag="sc")
                        for c in range(6):
                            nc.tensor.matmul(
                                ps[:, :fw], ln2T[:, c, 128 * g:128 * g + 128],
                                woutT[:, c, f0:f0 + fw],
                                start=(c == 0), stop=(c == 5))
                        if with_bias:
                            nc.vector.scalar_tensor_tensor(
                                out=ot[:, f0:f0 + fw], in0=ps[:, :fw], scalar=1.0,
                                in1=boutr[:, f0:f0 + fw], op0=OP.mult, op1=OP.add)
                        else:
                            nc.vector.tensor_copy(out=ot[:, f0:f0 + fw], in_=ps[:, :fw])
                    nc.sync.dma_start(out=out_d[128 * g:128 * g + 128, :], in_=ot[:])

            # software-pipelined driver: head i+1's expansions are emitted
            # before head i's score/PV matmuls so the shear+transpose DMA
            # latency hides behind real PE work.
            from collections import deque
            work = [(b, hh) for b in range(BL) for hh in range(NH)]
            pend = deque()
            fin = deque()
            def drain_fin():
                (fb_, fh_), fcps = fin.popleft()
                emit_ctx_finish(fb_, fh_, fcps)
                if fh_ == NH - 1:
                    emit_p4(fb_)
            def drain_one():
                (pb_, ph_), (pcqT, pcksh) = pend.popleft()
                cps = emit_scores_pv(pb_, ph_, pcqT, pcksh)
                fin.append(((pb_, ph_), cps))
                if len(fin) > 1:
                    drain_fin()
            for w in work:
                tiles = emit_expansions(*w)
                pend.append((w, tiles))
                if len(pend) > 2:
                    drain_one()
            while pend:
                drain_one()
            while fin:
                drain_fin()

    return nc


# ---------------------------------------------------------------------------
# host side
# ---------------------------------------------------------------------------
def _host_prep(position_indices, attention_mask):
    pi = np.asarray(position_indices)
    gvec = np.empty(1023, np.int64)
    gvec[511:] = pi[:, 0]
    gvec[:512] = pi[0, ::-1]
    d = np.arange(S)[:, None] - np.arange(S)[None, :]
    assert np.array_equal(gvec[d + 511], pi), "position_indices not Toeplitz"
    e = np.arange(1023)
    # table col c maps to window col j = c - ws; one zero col prepended so the
    # t=3 window slice starts at col 0 (cq: c = 512 - delta, ck: c = 512 + delta)
    E_cq = np.zeros((NB, EW), np.float16)
    E_ck = np.zeros((NB, EW), np.float16)
    E_cq[:, 1:1024] = (np.arange(NB)[:, None] == gvec[1022 - e][None, :])
    E_ck[:, 1:1024] = (np.arange(NB)[:, None] == gvec[e][None, :])
    am = np.asarray(attention_mask).reshape(B, S)
    vmask = (~am).astype(np.float32)
    return E_cq, E_ck, vmask


def kernel(hidden_states, relative_embedding, w_qk, b_qk, w_vg, b_vg,
           w_out, b_out, attention_mask, position_indices):
    from concourse.bass_utils import run_bass_kernel_spmd

    hidden_states = np.asarray(hidden_states, dtype=np.float32)
    relative_embedding = np.asarray(relative_embedding, dtype=np.float32)
    w_qk = np.asarray(w_qk, dtype=np.float32)
    w_vg = np.asarray(w_vg, dtype=np.float32)
    w_out = np.asarray(w_out, dtype=np.float32)
    b_qk = np.asarray(b_qk, dtype=np.float32)
    b_vg = np.asarray(b_vg, dtype=np.float32)
    b_out = np.asarray(b_out, dtype=np.float32)

    with_bias = bool(np.any(b_qk) or np.any(b_vg) or np.any(b_out))
    E_cq, E_ck, vmask = _host_prep(position_indices, attention_mask)

    nc = build_module(with_bias)
    common = dict(
        wqkT=np.ascontiguousarray(w_qk.T).astype(np.float16),
        wvgT=np.ascontiguousarray(w_vg.T).astype(np.float16),
        woutT=np.ascontiguousarray(w_out.T).astype(np.float16),
        relT=np.ascontiguousarray(relative_embedding.T).astype(np.float16),
        Ecq=E_cq, Eck=E_ck)
    if with_bias:
        sc_col = np.where(np.arange(12) < 6, SCALE, 1.0).astype(np.float32)
        common["bqkc"] = np.ascontiguousarray(
            b_qk.reshape(12, 128).T * sc_col[None, :])
        sc_row = np.concatenate([np.full(H, SCALE), np.ones(H)]).astype(np.float32)
        common["bqkr"] = (b_qk * sc_row)[None, :].astype(np.float32)
        common["bvgr"] = b_vg[None, :].astype(np.float32)
        common["boutr"] = b_out[None, :].astype(np.float32)

    in_maps = []
    for core in range(NCORES):
        bsel = [BL * core + i for i in range(BL)]
        hid = np.ascontiguousarray(
            hidden_states[:, bsel, :].transpose(1, 0, 2).reshape(T, H))
        vm = np.ascontiguousarray(vmask[bsel].reshape(T, 1))
        in_maps.append(dict(common, hid=hid, vmask=vm))

    res = run_bass_kernel_spmd(nc, in_maps, list(range(NCORES)))
    out = np.empty((S, B, H), np.float32)
    for core in range(NCORES):
        o = res.results[core]["out"].reshape(BL, S, H)
        for i in range(BL):
            out[:, BL * core + i, :] = o[i]
    return out
